# revision 1
# baseline (speedup 1.0000x reference)
"""ChannelAttention Trainium2 kernel (self-contained).

Problem: B=16, H=W=64 (N=4096 tokens), C=512, heads=8, d=64, fp32.
  qkv = x @ qkv_w (+bias);  q,k l2-normalized over tokens;
  attn = softmax((q*exp(scale))^T k);  out = attn @ v^T;  y = out @ proj_w + b.

Sharding: pure data-parallel, 2 batches per core on 8 cores. No collectives.

Per-core layout strategy (zero on-device transposes of big tensors):
  - host pre-transposes x to x^T [C, N] per batch, so the c-contraction dim is
    always the SBUF partition dim.
  - qkv_w is split/permuted on host into wqk ([c, (head, q|k, d)]) and
    wv ([c, (head, d)]); proj_w into [ci, cchunk, cout].
  - q/k are never normalized: the per-head Gram Z^T Z with Z=[q_h|k_h]
    (bf16) provides both q^T k and the l2 norms (diagonal). Normalization +
    exp(scale) fold into the 64x64 attention matrix pre-softmax.
  - big matmuls (qkv, v^T, attn@v^T, proj) run as float32r (TF32-like,
    full PE speed at moving-dim >= 256) on fp32 data.
"""

import os
import numpy as np

P = 128
C = 512
CCH = C // P            # 4 contraction chunks
HEADS = 8
NPAIR = HEADS // 2      # 4 head pairs
D = 64
EPS = 1.55e-5
N_CORES = 8

_CACHE = {}


def _pbroadcast(bass, ap, p):
    # read a [1, F] DRAM row with partition-step 0 -> broadcast to p partitions
    return bass.AP(tensor=ap.tensor, offset=ap.offset,
                   ap=[[0, p]] + [list(d) for d in ap.ap[1:]])


def _build(nb, n, es, add_bqk, add_bv, add_bp):
    """Build + compile the per-core Bass kernel.

    nb: batches per core; n: tokens per batch; es: tuple of 8 python floats
    (exp(scale), baked); add_*: whether bias adds are emitted.
    """
    from contextlib import ExitStack
    import concourse.bass as bass  # noqa: F401  (registers engine classes)
    from concourse import bacc
    import concourse.mybir as mybir
    import concourse.tile as tile
    from concourse.masks import make_identity

    f32 = mybir.dt.float32
    f32r = mybir.dt.float32r
    bf16 = mybir.dt.bfloat16
    X = mybir.AxisListType.X
    AF = mybir.ActivationFunctionType

    nt = n // P             # token tiles per batch
    nxc = n // 512          # 512-token x chunks per batch
    tiles_per_sc = min(8, nt)
    nsc = nt // tiles_per_sc  # super chunks (gram accumulation granularity)
    xc_per_sc = (512 * nxc) // (512 * nsc)  # x chunks per super chunk

    nc = bacc.Bacc("TRN2", target_bir_lowering=False)

    xt_d = nc.dram_tensor("xt", [nb, C, n], f32r, kind="ExternalInput")
    wqk_d = nc.dram_tensor("wqk", [P, CCH, 2 * C], f32r, kind="ExternalInput")
    wv_d = nc.dram_tensor("wv", [P, CCH, C], f32r, kind="ExternalInput")
    wp_d = nc.dram_tensor("wp", [P, CCH, C], f32r, kind="ExternalInput")
    y_d = nc.dram_tensor("y", [nb, n, C], f32, kind="ExternalOutput")
    if add_bqk:
        bqk_d = nc.dram_tensor("bqk", [1, 2 * C], f32, kind="ExternalInput")
    if add_bv:
        bv_d = nc.dram_tensor("bv", [C], f32, kind="ExternalInput")
    if add_bp:
        bp_d = nc.dram_tensor("bp", [1, C], f32, kind="ExternalInput")

    with tile.TileContext(nc) as tc, ExitStack() as ctx:
        consts = ctx.enter_context(tc.tile_pool(name="consts", bufs=1))
        vt_pool = ctx.enter_context(tc.tile_pool(name="vt", bufs=1))
        o2_pool = ctx.enter_context(tc.tile_pool(name="o2", bufs=1))
        x_pool = ctx.enter_context(tc.tile_pool(name="xp", bufs=2))
        z_pool = ctx.enter_context(tc.tile_pool(name="zp", bufs=min(9, nt + 1)))
        g_pool = ctx.enter_context(tc.tile_pool(name="gp", bufs=HEADS))
        at_pool = ctx.enter_context(tc.tile_pool(name="atp", bufs=2))
        sm_pool = ctx.enter_context(tc.tile_pool(name="smp", bufs=2))
        y_pool = ctx.enter_context(tc.tile_pool(name="yp", bufs=2))
        pqk = ctx.enter_context(tc.tile_pool(name="pqk", bufs=3, space="PSUM"))
        pgram = ctx.enter_context(tc.tile_pool(name="pgram", bufs=2, space="PSUM"))
        pmisc = ctx.enter_context(tc.tile_pool(name="pmisc", bufs=2, space="PSUM"))
        ptr = ctx.enter_context(tc.tile_pool(name="ptr", bufs=1, space="PSUM"))

        # --- resident constants ---
        wqk_sb = consts.tile([P, CCH, 2 * C], f32r)
        nc.sync.dma_start(wqk_sb[:], wqk_d[:])
        wv_sb = consts.tile([P, CCH, C], f32r)
        nc.sync.dma_start(wv_sb[:], wv_d[:])
        wp_sb = consts.tile([P, CCH, C], f32r)
        nc.sync.dma_start(wp_sb[:], wp_d[:])
        ident = consts.tile([P, P], f32)
        make_identity(nc, ident[:])
        # ident_off: rows 64+j have 1 at col j (for building diag(s_k) at base 64)
        ioff = consts.tile([P, D], f32)
        nc.gpsimd.memset(ioff[:], 0.0)
        nc.gpsimd.affine_select(
            out=ioff[:], in_=ioff[:], compare_op=mybir.AluOpType.not_equal,
            fill=1.0, base=-D, pattern=[[-1, D]], channel_multiplier=1,
        )
        if add_bqk:
            bqk_sb = consts.tile([P, 2 * C], f32)
            nc.sync.dma_start(
                out=bqk_sb[:],
                in_=_pbroadcast(bass, bqk_d[:], P),
            )
        if add_bv:
            bv_sb = consts.tile([P, NPAIR], f32)
            nc.sync.dma_start(
                out=bv_sb[:], in_=bv_d[:].rearrange("(g p) -> p g", p=P))
        if add_bp:
            bp_sb = consts.tile([P, C], f32)
            nc.sync.dma_start(
                out=bp_sb[:],
                in_=_pbroadcast(bass, bp_d[:], P),
            )

        for b in range(nb):
            # ---- phase 1: qkv generation + per-head Gram accumulation ----
            vt = vt_pool.tile([P, NPAIR, n], f32r, tag="vt")
            gsb = [g_pool.tile([P, P], f32, tag="g", name=f"gsb{b}_{h}")
                   for h in range(HEADS)]
            xt_r = xt_d[b].rearrange("(co ci) n -> ci co n", ci=P)

            for sc in range(nsc):
                zs = []
                for xc in range(xc_per_sc):
                    tch = sc * xc_per_sc + xc
                    xt_t = x_pool.tile([P, CCH, 512], f32r, tag="x")
                    nc.sync.dma_start(
                        out=xt_t[:], in_=xt_r[:, :, tch * 512:(tch + 1) * 512])
                    # v^T for this 512-token chunk (pair-major feature rows)
                    for f in range(NPAIR):
                        pv = pmisc.tile([P, 512], f32, tag="pm")
                        for c in range(CCH):
                            nc.tensor.matmul(
                                pv[:],
                                wv_sb[:, c, f * P:(f + 1) * P],
                                xt_t[:, c, :],
                                start=(c == 0), stop=(c == CCH - 1),
                            )
                        dst = vt[:, f, tch * 512:(tch + 1) * 512]
                        if add_bv:
                            nc.vector.tensor_scalar(
                                out=dst, in0=pv[:], scalar1=bv_sb[:, f:f + 1],
                                scalar2=None, op0=mybir.AluOpType.add)
                        else:
                            nc.vector.tensor_copy(out=dst, in_=pv[:])
                    # q|k features for the 4 token tiles of this chunk
                    for t4 in range(4):
                        z = z_pool.tile([P, 2 * C], bf16, tag="z")
                        for fc in range(2):
                            pq = pqk.tile([P, 512], f32, tag="pq")
                            for c in range(CCH):
                                nc.tensor.matmul(
                                    pq[:],
                                    xt_t[:, c, t4 * P:(t4 + 1) * P],
                                    wqk_sb[:, c, fc * 512:(fc + 1) * 512],
                                    start=(c == 0), stop=(c == CCH - 1),
                                )
                            zdst = z[:, fc * 512:(fc + 1) * 512]
                            if add_bqk:
                                nc.vector.tensor_add(
                                    out=zdst, in0=pq[:],
                                    in1=bqk_sb[:, fc * 512:(fc + 1) * 512])
                            else:
                                nc.vector.tensor_copy(out=zdst, in_=pq[:])
                        zs.append(z)
                # per-head Gram contribution of this super chunk
                for h in range(HEADS):
                    pg = pgram.tile([P, P], f32, tag="pg")
                    for i, z in enumerate(zs):
                        zh = z[:, h * P:(h + 1) * P]
                        nc.tensor.matmul(
                            pg[:], zh, zh,
                            start=(i == 0), stop=(i == len(zs) - 1))
                    if sc == 0:
                        nc.vector.tensor_copy(out=gsb[h][:], in_=pg[:])
                    else:
                        nc.vector.tensor_add(
                            out=gsb[h][:], in0=gsb[h][:], in1=pg[:])

            # ---- phase 2: per-head softmax'd attention, transposed ----
            o2 = o2_pool.tile([P, NPAIR, n], f32r, tag="o2")
            for g in range(NPAIR):
                tin = sm_pool.tile([P, P], f32, tag="tin")
                nc.vector.memset(tin[:], 0.0)
                for hh in range(2):
                    h = 2 * g + hh
                    G = gsb[h]
                    # diagonal of the Gram = squared l2 norms of q|k columns
                    dtmp = sm_pool.tile([P, P], f32, tag="dtmp")
                    nc.vector.tensor_mul(dtmp[:], G[:], ident[:])
                    s = sm_pool.tile([P, 1], f32, tag="s")
                    nc.vector.reduce_sum(out=s[:], in_=dtmp[:], axis=X)
                    nc.vector.tensor_scalar_max(out=s[:], in0=s[:], scalar1=EPS)
                    srt = sm_pool.tile([P, 1], f32, tag="srt")
                    nc.scalar.activation(out=srt[:], in_=s[:], func=AF.Sqrt)
                    nc.vector.reciprocal(out=s[:], in_=srt[:])
                    # fold exp(scale_h) into the q-side norms
                    if es[h] != 1.0:
                        nc.scalar.mul(out=s[0:D, :], in_=s[0:D, :], mul=es[h])
                    # diag(s_k) at partition base 64
                    dsk = sm_pool.tile([P, D], f32, tag="dsk")
                    nc.vector.tensor_scalar_mul(
                        out=dsk[D:P, :], in0=ioff[D:P, :], scalar1=s[D:P, :])
                    # attn_pre[dd, e] = (q^T k)[dd, e] * s_k[e]   (tiny matmul)
                    pa = ptr.tile([P, P], f32, tag="pt")
                    nc.tensor.matmul(
                        pa[0:D, 0:D],
                        G[D:P, 0:D],
                        dsk[D:P, :],
                        start=True, stop=True,
                    )
                    # * s_q[dd]*es  on eviction
                    asb = sm_pool.tile([D, D], f32, tag="asb")
                    nc.vector.tensor_scalar_mul(
                        out=asb[:], in0=pa[0:D, 0:D], scalar1=s[0:D, :])
                    # softmax over e (free dim)
                    nm = sm_pool.tile([D, 1], f32, tag="nm")
                    nc.vector.tensor_reduce(
                        out=nm[:], in_=asb[:], op=mybir.AluOpType.max,
                        axis=X, negate=True)
                    ex = sm_pool.tile([D, D], f32, tag="ex")
                    zsum = sm_pool.tile([D, 1], f32, tag="zsum")
                    nc.scalar.activation(
                        out=ex[:], in_=asb[:], func=AF.Exp,
                        bias=nm[:], scale=1.0, accum_out=zsum[:])
                    rinv = sm_pool.tile([D, 1], f32, tag="rinv")
                    nc.vector.reciprocal(out=rinv[:], in_=zsum[:])
                    nc.vector.tensor_scalar_mul(
                        out=tin[hh * D:(hh + 1) * D, hh * D:(hh + 1) * D],
                        in0=ex[:], scalar1=rinv[:])
                # transpose the block-diagonal 2-head attn
                pt = ptr.tile([P, P], f32, tag="pt")
                nc.tensor.transpose(pt[:], tin[:], ident[:])
                at2 = at_pool.tile([P, P], f32r, tag="at")
                nc.vector.tensor_copy(out=at2[:], in_=pt[:])
                # ---- phase 3: out^T = attn @ v^T for this head pair ----
                for ch in range(n // 512):
                    po = pmisc.tile([P, 512], f32, tag="pm")
                    nc.tensor.matmul(
                        po[:],
                        at2[:],
                        vt[:, g, ch * 512:(ch + 1) * 512],
                        start=True, stop=True,
                    )
                    nc.scalar.copy(
                        out=o2[:, g, ch * 512:(ch + 1) * 512], in_=po[:])

            # ---- phase 4: projection ----
            for tt in range(nt):
                py = pmisc.tile([P, 512], f32, tag="pm")
                for g in range(NPAIR):
                    nc.tensor.matmul(
                        py[:],
                        o2[:, g, tt * P:(tt + 1) * P],
                        wp_sb[:, g, :],
                        start=(g == 0), stop=(g == NPAIR - 1),
                    )
                ysb = y_pool.tile([P, C], f32, tag="y")
                if add_bp:
                    nc.vector.tensor_add(out=ysb[:], in0=py[:], in1=bp_sb[:])
                else:
                    nc.vector.tensor_copy(out=ysb[:], in_=py[:])
                nc.sync.dma_start(
                    out=y_d[b, tt * P:(tt + 1) * P, :], in_=ysb[:])

    nc.compile()
    return nc


def _get_nc(nb, n, es, add_bqk, add_bv, add_bp):
    key = (nb, n, es, add_bqk, add_bv, add_bp)
    if key not in _CACHE:
        _CACHE[key] = _build(nb, n, es, add_bqk, add_bv, add_bp)
    return _CACHE[key]


def prep_inputs(x, qkv_w, q_bias, v_bias, scale, proj_w, proj_b, n_cores=N_CORES):
    """Host-side shard + layout prep. Returns (in_maps, es, gates, meta)."""
    B, H, W, Cc = x.shape
    assert Cc == C
    n = H * W
    nb = B // n_cores

    # x^T per batch: [B, C, N]
    xt = np.ascontiguousarray(
        x.reshape(B, n, C).transpose(0, 2, 1)).astype(np.float32, copy=False)

    w3 = qkv_w.reshape(C, HEADS, 3, D)
    wqk = np.ascontiguousarray(w3[:, :, 0:2, :].reshape(C, 2 * C))
    wv = np.ascontiguousarray(w3[:, :, 2, :].reshape(C, C))
    # [c, f] -> [ci, cchunk, f]
    wqk = np.ascontiguousarray(wqk.reshape(CCH, P, 2 * C).transpose(1, 0, 2))
    wv = np.ascontiguousarray(wv.reshape(CCH, P, C).transpose(1, 0, 2))
    wp = np.ascontiguousarray(proj_w.reshape(CCH, P, C).transpose(1, 0, 2))

    # biases exactly as the reference applies them: concat([q_bias, 0, v_bias])
    # indexed by the raw qkv feature id, then split/permuted like the weights
    bias_full = np.concatenate(
        [q_bias, np.zeros_like(q_bias), v_bias]).astype(np.float32)
    b3 = bias_full.reshape(HEADS, 3, D)
    bqk = np.ascontiguousarray(b3[:, 0:2, :].reshape(1, 2 * C))
    bv = np.ascontiguousarray(b3[:, 2, :].reshape(C))
    bp = np.asarray(proj_b, np.float32).reshape(1, C)

    add_bqk = bool(np.any(bqk))
    add_bv = bool(np.any(bv))
    add_bp = bool(np.any(bp))
    es = tuple(float(v) for v in
               np.exp(np.asarray(scale, np.float32)).reshape(HEADS))

    in_maps = []
    for core in range(n_cores):
        m = {
            "xt": np.ascontiguousarray(xt[core * nb:(core + 1) * nb]),
            "wqk": wqk, "wv": wv, "wp": wp,
        }
        if add_bqk:
            m["bqk"] = bqk
        if add_bv:
            m["bv"] = bv
        if add_bp:
            m["bp"] = bp
        in_maps.append(m)
    return in_maps, es, (add_bqk, add_bv, add_bp), (B, H, W, nb, n)


def kernel(x, qkv_w, q_bias, v_bias, scale, proj_w, proj_b):
    from concourse.bass_utils import run_bass_kernel_spmd

    in_maps, es, gates, (B, H, W, nb, n) = prep_inputs(
        x, qkv_w, q_bias, v_bias, scale, proj_w, proj_b)
    nc = _get_nc(nb, n, es, *gates)
    res = run_bass_kernel_spmd(
        nc, in_maps, core_ids=list(range(N_CORES)),
        trace=bool(int(os.environ.get("KERNEL_TRACE", "0"))),
    )
    y = np.concatenate([r["y"] for r in res.results], axis=0)
    out = y.reshape(B, H, W, C).astype(np.float32, copy=False)
    kernel.last_results = res
    return out



# revision 7
# speedup vs baseline: 1.4283x; 1.4283x over previous
"""ChannelAttention Trainium2 kernel (self-contained).

Problem: B=16, H=W=64 (N=4096 tokens), C=512, heads=8, d=64, fp32.
  qkv = x @ qkv_w (+bias);  q,k l2-normalized over tokens;
  attn = softmax((q*exp(scale))^T k);  out = attn @ v^T;  y = out @ proj_w + b.

Sharding: pure data-parallel, 2 batches per core on 8 cores. No collectives.

Algorithm (per batch) — restructured to halve the matmul FLOPs vs the
direct formulation:
  1. G = x^T x                      [C, C]    (contract over N tokens)
  2. T = G @ W_qk                   [C, 2C]   (W_qk = per-head [q|k] columns)
  3. A_h = W_qk_h^T T_h             [128,128] per head = Gram of [q_h|k_h]
     -> diag gives the l2 norms, off-diag block gives q^T k.
     softmax machinery identical to the direct Gram formulation.
  4. W_eff = sum_h W_v_h attn_h^T W_p_h   [C, C]  (head-pair-stacked matmuls)
  5. y^T = W_eff^T x^T              [C, N]   (host transposes back)
Biases (zero in this problem) are handled via gated correction terms.

Layouts: x ships twice — token-major fp32 (for G) and channel-major bf16
(for step 5). y returns transposed; host does the final [C,N]->[N,C].
"""

import os
import numpy as np

P = 128
C = 512
CCH = C // P            # 4 channel tiles
HEADS = 8
NPAIR = HEADS // 2      # 4 head pairs
D = 64
EPS = 1.55e-5
N_CORES = 8

_CACHE = {}


def _build(nb, n, es, add_acorr, add_bv, add_bp):
    """Build + compile the per-core Bass kernel.

    nb: batches per core; n: tokens per batch; es: tuple of 8 python floats
    (exp(scale), baked); add_*: whether bias corrections are emitted.
    """
    from contextlib import ExitStack
    import concourse.bass as bass  # noqa: F401  (registers engine classes)
    from concourse import bacc
    import concourse.mybir as mybir
    import concourse.tile as tile
    from concourse.masks import make_identity

    f32 = mybir.dt.float32
    f32r = mybir.dt.float32r
    bf16 = mybir.dt.bfloat16
    X = mybir.AxisListType.X
    AF = mybir.ActivationFunctionType

    nt = n // P             # token tiles per batch (32)
    ndma = nt // 4          # x DMAs per batch (4 token tiles each)
    nch = n // 512          # 512-token chunks per batch (8)

    nc = bacc.Bacc("TRN2", target_bir_lowering=False)

    x_d = nc.dram_tensor("x", [nb, n, C], f32r, kind="ExternalInput")
    xt_d = nc.dram_tensor("xt", [nb, C, n], bf16, kind="ExternalInput")
    wqk_d = nc.dram_tensor("wqk", [P, CCH, 2 * C], f32r, kind="ExternalInput")
    wvt_d = nc.dram_tensor("wvt", [P, NPAIR, C], f32r, kind="ExternalInput")
    wp_d = nc.dram_tensor("wp", [P, NPAIR, C], f32r, kind="ExternalInput")
    yt_d = nc.dram_tensor("yt", [nb, C, n], f32, kind="ExternalOutput")
    if add_acorr:
        acorr_d = nc.dram_tensor(
            "acorr", [nb, HEADS, P, P], f32, kind="ExternalInput")
    if add_bv:
        bv_d = nc.dram_tensor("bv", [P, NPAIR], f32, kind="ExternalInput")
    if add_bp:
        bp_d = nc.dram_tensor("bp", [P, CCH], f32, kind="ExternalInput")

    with tile.TileContext(nc) as tc, ExitStack() as ctx:
        consts = ctx.enter_context(tc.tile_pool(name="consts", bufs=1))
        x_pool = ctx.enter_context(tc.tile_pool(name="xp", bufs=3))
        xt_pool = ctx.enter_context(tc.tile_pool(name="xtp", bufs=2))
        g_pool = ctx.enter_context(tc.tile_pool(name="gp", bufs=2))
        t_pool = ctx.enter_context(tc.tile_pool(name="tp", bufs=2))
        u_pool = ctx.enter_context(tc.tile_pool(name="up", bufs=NPAIR + 1))
        w_pool = ctx.enter_context(tc.tile_pool(name="wp", bufs=2))
        y_pool = ctx.enter_context(tc.tile_pool(name="yp", bufs=3))
        sm_pool = ctx.enter_context(tc.tile_pool(name="smp", bufs=2))
        ac_pool = ctx.enter_context(tc.tile_pool(name="acp", bufs=2))
        pgram = ctx.enter_context(tc.tile_pool(name="pgram", bufs=4, space="PSUM"))
        pmm = ctx.enter_context(tc.tile_pool(name="pmm", bufs=2, space="PSUM"))
        pw = ctx.enter_context(tc.tile_pool(name="pw", bufs=2, space="PSUM"))

        # --- resident constants ---
        wqk_sb = consts.tile([P, CCH, 2 * C], f32r)
        nc.sync.dma_start(wqk_sb[:], wqk_d[:])
        wvt_sb = consts.tile([P, NPAIR, C], f32r)
        nc.sync.dma_start(wvt_sb[:], wvt_d[:])
        wp_sb = consts.tile([P, NPAIR, C], f32r)
        nc.sync.dma_start(wp_sb[:], wp_d[:])
        ident = consts.tile([P, P], f32)
        make_identity(nc, ident[:])
        zero128 = consts.tile([P, P], f32)
        nc.vector.memset(zero128[:], 0.0)
        # ident_off: rows 64+j have 1 at col j (for building diag(s_k))
        ioff = consts.tile([P, D], f32)
        nc.gpsimd.memset(ioff[:], 0.0)
        nc.gpsimd.affine_select(
            out=ioff[:], in_=ioff[:], compare_op=mybir.AluOpType.not_equal,
            fill=1.0, base=-D, pattern=[[-1, D]], channel_multiplier=1,
        )
        if add_bv:
            bv_sb = consts.tile([P, NPAIR], f32)
            nc.sync.dma_start(out=bv_sb[:], in_=bv_d[:])
        if add_bp:
            bp_sb = consts.tile([P, CCH], f32)
            nc.sync.dma_start(out=bp_sb[:], in_=bp_d[:])

        for b in range(nb):
            x_r = x_d[b].rearrange("(nt p) c -> p nt c", p=P)
            xt_r = xt_d[b].rearrange("(co p) n -> p co n", p=P)
            yt_r = yt_d[b].rearrange("(ft p) n -> p ft n", p=P)

            # ---- phase 1: G = x^T x  (accumulate over token tiles) ----
            gps = [pgram.tile([P, C], f32, tag="g", name=f"g{b}_{co}")
                   for co in range(CCH)]
            for dd in range(ndma):
                x_t = x_pool.tile([P, 4, C], f32r, tag="x")
                nc.sync.dma_start(out=x_t[:], in_=x_r[:, dd * 4:(dd + 1) * 4, :])
                for tt in range(4):
                    t = dd * 4 + tt
                    for co in range(CCH):
                        nc.tensor.matmul(
                            gps[co][:],
                            x_t[:, tt, co * P:(co + 1) * P],
                            x_t[:, tt, :],
                            start=(t == 0), stop=(t == nt - 1),
                        )
            g_sb = g_pool.tile([P, CCH, C], f32r, tag="g")
            for co in range(CCH):
                nc.vector.tensor_copy(out=g_sb[:, co, :], in_=gps[co][:])

            # ---- phase 2: T = G @ W_qk ----
            t_sb = t_pool.tile([P, CCH, 2 * C], f32r, tag="t")
            for fc in range(2):
                for j in range(CCH):
                    pt = pmm.tile([P, C], f32, tag="pm")
                    for i in range(CCH):
                        nc.tensor.matmul(
                            pt[:],
                            g_sb[:, i, j * P:(j + 1) * P],
                            wqk_sb[:, i, fc * C:(fc + 1) * C],
                            start=(i == 0), stop=(i == CCH - 1),
                        )
                    nc.vector.tensor_copy(
                        out=t_sb[:, j, fc * C:(fc + 1) * C], in_=pt[:])

            # ---- phase 3: per-head A = Gram of [q|k]; softmax; U ----
            u_sbs = []
            for g in range(NPAIR):
                tin = sm_pool.tile([P, P], f32r, tag="tin")
                nc.vector.tensor_copy(out=tin[:], in_=zero128[:])
                for hh in range(2):
                    h = 2 * g + hh
                    pa = pmm.tile([P, P], f32, tag="pm")
                    for i in range(CCH):
                        nc.tensor.matmul(
                            pa[:],
                            wqk_sb[:, i, h * P:(h + 1) * P],
                            t_sb[:, i, h * P:(h + 1) * P],
                            start=(i == 0), stop=(i == CCH - 1),
                        )
                    A = sm_pool.tile([P, P], f32, tag="A")
                    if add_acorr:
                        ac = ac_pool.tile([P, P], f32, tag="ac")
                        nc.sync.dma_start(out=ac[:], in_=acorr_d[b, h])
                        nc.vector.tensor_add(out=A[:], in0=pa[:], in1=ac[:])
                    else:
                        nc.vector.tensor_copy(out=A[:], in_=pa[:])
                    # diagonal of A = squared l2 norms of q|k columns
                    dtmp = sm_pool.tile([P, P], f32, tag="dtmp")
                    nc.vector.tensor_mul(dtmp[:], A[:], ident[:])
                    s = sm_pool.tile([P, 1], f32, tag="s")
                    nc.vector.reduce_sum(out=s[:], in_=dtmp[:], axis=X)
                    nc.vector.tensor_scalar_max(out=s[:], in0=s[:], scalar1=EPS)
                    srt = sm_pool.tile([P, 1], f32, tag="srt")
                    nc.scalar.activation(out=srt[:], in_=s[:], func=AF.Sqrt)
                    nc.vector.reciprocal(out=s[:], in_=srt[:])
                    # fold exp(scale_h) into the q-side norms
                    if es[h] != 1.0:
                        nc.scalar.mul(out=s[0:D, :], in_=s[0:D, :], mul=es[h])
                    # diag(s_k) at partition base 64
                    dsk = sm_pool.tile([P, D], f32, tag="dsk")
                    nc.vector.tensor_scalar_mul(
                        out=dsk[D:P, :], in0=ioff[D:P, :], scalar1=s[D:P, :])
                    # attn_pre[dd, e] = (q^T k)[dd, e] * s_k[e]   (tiny matmul)
                    pa2 = pmm.tile([P, P], f32, tag="pm")
                    nc.tensor.matmul(
                        pa2[0:D, 0:D],
                        A[D:P, 0:D],
                        dsk[D:P, :],
                        start=True, stop=True,
                    )
                    # * s_q[dd]*es  on eviction
                    asb = sm_pool.tile([D, D], f32, tag="asb")
                    nc.vector.tensor_scalar_mul(
                        out=asb[:], in0=pa2[0:D, 0:D], scalar1=s[0:D, :])
                    # softmax over e (free dim)
                    nm = sm_pool.tile([D, 1], f32, tag="nm")
                    nc.vector.tensor_reduce(
                        out=nm[:], in_=asb[:], op=mybir.AluOpType.max,
                        axis=X, negate=True)
                    ex = sm_pool.tile([D, D], f32, tag="ex")
                    zsum = sm_pool.tile([D, 1], f32, tag="zsum")
                    nc.scalar.activation(
                        out=ex[:], in_=asb[:], func=AF.Exp,
                        bias=nm[:], scale=1.0, accum_out=zsum[:])
                    rinv = sm_pool.tile([D, 1], f32, tag="rinv")
                    nc.vector.reciprocal(out=rinv[:], in_=zsum[:])
                    nc.vector.tensor_scalar_mul(
                        out=tin[hh * D:(hh + 1) * D, hh * D:(hh + 1) * D],
                        in0=ex[:], scalar1=rinv[:])
                # U_pair = attn_pair^T @ W_p_pair   [e-stack, C]
                pu = pmm.tile([P, C], f32, tag="pm")
                nc.tensor.matmul(
                    pu[:], tin[:], wp_sb[:, g, :], start=True, stop=True)
                u_sb = u_pool.tile([P, C], f32r, tag="u", name=f"u{b}_{g}")
                nc.vector.tensor_copy(out=u_sb[:], in_=pu[:])
                u_sbs.append(u_sb)

            # ---- phase 4: W_eff = sum_g W_v_pair^T-stack @ U_pair ----
            weff_sb = w_pool.tile([P, CCH, C], bf16, tag="weff")
            for j in range(CCH):
                pwj = pw.tile([P, C], f32, tag="pw")
                for g in range(NPAIR):
                    nc.tensor.matmul(
                        pwj[:],
                        wvt_sb[:, g, j * P:(j + 1) * P],
                        u_sbs[g][:],
                        start=(g == 0), stop=(g == NPAIR - 1),
                    )
                nc.vector.tensor_copy(out=weff_sb[:, j, :], in_=pwj[:])

            # bias row r[f] = sum_e bv[e] U[e, f] (+ proj_b): gated
            if add_bv or add_bp:
                rfull = sm_pool.tile([P, CCH], f32, tag="rf")
                if add_bv:
                    pr = pmm.tile([P, CCH], f32, tag="pm")
                    for jf in range(CCH):
                        for g in range(NPAIR):
                            nc.tensor.matmul(
                                pr[:, jf:jf + 1],
                                u_sbs[g][:, jf * P:(jf + 1) * P],
                                bv_sb[:, g:g + 1],
                                start=(g == 0), stop=(g == NPAIR - 1),
                            )
                    if add_bp:
                        nc.vector.tensor_add(
                            out=rfull[:], in0=pr[:], in1=bp_sb[:])
                    else:
                        nc.vector.tensor_copy(out=rfull[:], in_=pr[:])
                else:
                    nc.vector.tensor_copy(out=rfull[:], in_=bp_sb[:])

            # ---- phase 5: y^T = W_eff^T @ x^T ----
            for ch in range(nch):
                xT_t = xt_pool.tile([P, CCH, 512], bf16, tag="xT")
                nc.sync.dma_start(
                    out=xT_t[:], in_=xt_r[:, :, ch * 512:(ch + 1) * 512])
                y_sb = y_pool.tile([P, CCH, 512], f32, tag="y")
                for ft in range(CCH):
                    py = pmm.tile([P, 512], f32, tag="pm")
                    for i in range(CCH):
                        nc.tensor.matmul(
                            py[:],
                            weff_sb[:, i, ft * P:(ft + 1) * P],
                            xT_t[:, i, :],
                            start=(i == 0), stop=(i == CCH - 1),
                        )
                    if add_bv or add_bp:
                        nc.vector.tensor_scalar(
                            out=y_sb[:, ft, :], in0=py[:],
                            scalar1=rfull[:, ft:ft + 1], scalar2=None,
                            op0=mybir.AluOpType.add)
                    else:
                        nc.vector.tensor_copy(out=y_sb[:, ft, :], in_=py[:])
                nc.sync.dma_start(
                    out=yt_r[:, :, ch * 512:(ch + 1) * 512], in_=y_sb[:])

    nc.compile()
    return nc


def _get_nc(nb, n, es, add_acorr, add_bv, add_bp):
    key = (nb, n, es, add_acorr, add_bv, add_bp)
    if key not in _CACHE:
        _CACHE[key] = _build(nb, n, es, add_acorr, add_bv, add_bp)
    return _CACHE[key]


def prep_inputs(x, qkv_w, q_bias, v_bias, scale, proj_w, proj_b,
                n_cores=N_CORES):
    """Host-side shard + layout prep. Returns (in_maps, es, gates, meta)."""
    import ml_dtypes

    B, H, W, Cc = x.shape
    assert Cc == C
    n = H * W
    nb = B // n_cores

    xf = np.asarray(x, np.float32).reshape(B, n, C)
    # channel-major bf16 copy for the y = x @ W_eff pass
    xt = np.ascontiguousarray(xf.transpose(0, 2, 1)).astype(ml_dtypes.bfloat16)

    w3 = np.asarray(qkv_w, np.float32).reshape(C, HEADS, 3, D)
    wqk = np.ascontiguousarray(w3[:, :, 0:2, :].reshape(C, 2 * C))
    wqk_r = np.ascontiguousarray(
        wqk.reshape(CCH, P, 2 * C).transpose(1, 0, 2))
    wv = w3[:, :, 2, :].reshape(C, C)
    wvt_r = np.ascontiguousarray(
        np.ascontiguousarray(wv.T).reshape(NPAIR, P, C).transpose(1, 0, 2))
    wp_r = np.ascontiguousarray(
        np.asarray(proj_w, np.float32).reshape(NPAIR, P, C).transpose(1, 0, 2))

    # biases exactly as the reference applies them: concat([q_bias, 0, v_bias])
    # indexed by the raw qkv feature id
    bias_full = np.concatenate(
        [q_bias, np.zeros_like(q_bias), v_bias]).astype(np.float32)
    b3 = bias_full.reshape(HEADS, 3, D)
    bqk = b3[:, 0:2, :].reshape(HEADS, P)      # per-head [q|k] bias
    bv = b3[:, 2, :].reshape(C)                # v bias, (h, d) indexed
    bp = np.asarray(proj_b, np.float32).reshape(C)

    add_acorr = bool(np.any(bqk))
    add_bv = bool(np.any(bv))
    add_bp = bool(np.any(bp))
    es = tuple(float(v) for v in
               np.exp(np.asarray(scale, np.float32)).reshape(HEADS))

    acorr = None
    if add_acorr:
        # A_h correction for qkv bias: with z = [q|k] = x W_h + 1 b_h^T,
        # z^T z = W^T G W + b m^T + m b^T + N b b^T,  m = (1^T x) W_h
        sx = xf.sum(axis=1)                    # [B, C]
        wqk_h = wqk.reshape(C, HEADS, P)       # [c, h, f]
        m = np.einsum("bc,chf->bhf", sx, wqk_h)  # [B, HEADS, 128]
        acorr = (m[:, :, None, :] * bqk[None, :, :, None]
                 + m[:, :, :, None] * bqk[None, :, None, :]
                 + float(n) * bqk[None, :, :, None] * bqk[None, :, None, :]
                 ).astype(np.float32)          # [B, HEADS, 128, 128]
    bv_t = np.ascontiguousarray(
        bv.reshape(NPAIR, P).T).astype(np.float32) if add_bv else None
    bp_t = np.ascontiguousarray(
        bp.reshape(CCH, P).T).astype(np.float32) if add_bp else None

    in_maps = []
    for core in range(n_cores):
        sl = slice(core * nb, (core + 1) * nb)
        m_ = {
            "x": np.ascontiguousarray(xf[sl]),
            "xt": np.ascontiguousarray(xt[sl]),
            "wqk": wqk_r, "wvt": wvt_r, "wp": wp_r,
        }
        if add_acorr:
            m_["acorr"] = np.ascontiguousarray(acorr[sl])
        if add_bv:
            m_["bv"] = bv_t
        if add_bp:
            m_["bp"] = bp_t
        in_maps.append(m_)
    return in_maps, es, (add_acorr, add_bv, add_bp), (B, H, W, nb, n)


def kernel(x, qkv_w, q_bias, v_bias, scale, proj_w, proj_b):
    from concourse.bass_utils import run_bass_kernel_spmd

    in_maps, es, gates, (B, H, W, nb, n) = prep_inputs(
        x, qkv_w, q_bias, v_bias, scale, proj_w, proj_b)
    nc = _get_nc(nb, n, es, *gates)
    res = run_bass_kernel_spmd(
        nc, in_maps, core_ids=list(range(N_CORES)),
        trace=bool(int(os.environ.get("KERNEL_TRACE", "0"))),
    )
    yt = np.concatenate([r["yt"] for r in res.results], axis=0)  # [B, C, n]
    out = np.ascontiguousarray(yt.transpose(0, 2, 1)).reshape(B, H, W, C)
    kernel.last_results = res
    return out.astype(np.float32, copy=False)


# revision 11
# speedup vs baseline: 1.9146x; 1.3405x over previous
"""ChannelAttention Trainium2 kernel (self-contained).

Problem: B=16, H=W=64 (N=4096 tokens), C=512, heads=8, d=64, fp32.
  qkv = x @ qkv_w (+bias);  q,k l2-normalized over tokens;
  attn = softmax((q*exp(scale))^T k);  out = attn @ v^T;  y = out @ proj_w + b.

Sharding: pure data-parallel, 2 batches per core on 8 cores. No collectives.

Algorithm (per batch) — restructured to halve the matmul FLOPs vs the
direct formulation:
  1. G = x^T x                      [C, C]    (contract over N tokens)
  2. T = G @ W_qk                   [C, 2C]   (W_qk = per-head [q|k] columns)
  3. A_h = W_qk_h^T T_h             [128,128] per head = Gram of [q_h|k_h]
     -> diag gives the l2 norms, off-diag block gives q^T k.
     softmax machinery identical to the direct Gram formulation.
  4. W_eff = sum_h W_v_h attn_h^T W_p_h   [C, C]  (head-pair-stacked matmuls)
  5. y^T = W_eff^T x^T              [C, N]   (host transposes back)
Biases (zero in this problem) are handled via gated correction terms.

Layouts: x ships twice — token-major fp32 (for G) and channel-major bf16
(for step 5). y returns transposed; host does the final [C,N]->[N,C].
"""

import os
import numpy as np

P = 128
C = 512
CCH = C // P            # 4 channel tiles
HEADS = 8
NPAIR = HEADS // 2      # 4 head pairs
D = 64
EPS = 1.55e-5
N_CORES = 8

_CACHE = {}


def _build(nb, n, es, add_acorr, add_bv, add_bp):
    """Build + compile the per-core Bass kernel.

    nb: batches per core; n: tokens per batch; es: tuple of 8 python floats
    (exp(scale), baked); add_*: whether bias corrections are emitted.
    """
    from contextlib import ExitStack
    import concourse.bass as bass  # noqa: F401  (registers engine classes)
    from concourse import bacc
    import concourse.mybir as mybir
    import concourse.tile as tile
    from concourse.masks import make_identity

    f32 = mybir.dt.float32
    f32r = mybir.dt.float32r
    bf16 = mybir.dt.bfloat16
    X = mybir.AxisListType.X
    AF = mybir.ActivationFunctionType

    nt = n // P             # token tiles per batch (32)
    ndma = nt // 4          # x DMAs per batch (4 token tiles each)
    nch = n // 512          # 512-token chunks per batch (8)

    nc = bacc.Bacc("TRN2", target_bir_lowering=False)

    x_d = nc.dram_tensor("x", [nb, n, C], f32r, kind="ExternalInput")
    xt_d = nc.dram_tensor("xt", [nb, C, n], bf16, kind="ExternalInput")
    wqk_d = nc.dram_tensor("wqk", [P, CCH, 2 * C], f32r, kind="ExternalInput")
    wvt_d = nc.dram_tensor("wvt", [P, NPAIR, C], f32r, kind="ExternalInput")
    wp_d = nc.dram_tensor("wp", [P, NPAIR, C], f32r, kind="ExternalInput")
    yt_d = nc.dram_tensor("yt", [nb, C, n], f32, kind="ExternalOutput")
    if add_acorr:
        acorr_d = nc.dram_tensor(
            "acorr", [nb, HEADS, P, P], f32, kind="ExternalInput")
    if add_bv:
        bv_d = nc.dram_tensor("bv", [P, NPAIR], f32, kind="ExternalInput")
    if add_bp:
        bp_d = nc.dram_tensor("bp", [P, CCH], f32, kind="ExternalInput")

    with tile.TileContext(nc) as tc, ExitStack() as ctx:
        consts = ctx.enter_context(tc.tile_pool(name="consts", bufs=1))
        x_pool = ctx.enter_context(tc.tile_pool(name="xp", bufs=3))
        xt_pool = ctx.enter_context(tc.tile_pool(name="xtp", bufs=2))
        g_pool = ctx.enter_context(tc.tile_pool(name="gp", bufs=2))
        t_pool = ctx.enter_context(tc.tile_pool(name="tp", bufs=2))
        u_pool = ctx.enter_context(tc.tile_pool(name="up", bufs=NPAIR + 1))
        w_pool = ctx.enter_context(tc.tile_pool(name="wp", bufs=2))
        y_pool = ctx.enter_context(tc.tile_pool(name="yp", bufs=3))
        sm_pool = ctx.enter_context(tc.tile_pool(name="smp", bufs=2))
        a_pool = ctx.enter_context(tc.tile_pool(name="ap", bufs=2))
        ac_pool = ctx.enter_context(tc.tile_pool(name="acp", bufs=2))
        pgram = ctx.enter_context(tc.tile_pool(name="pgram", bufs=4, space="PSUM"))
        pmm = ctx.enter_context(tc.tile_pool(name="pmm", bufs=2, space="PSUM"))
        pw = ctx.enter_context(tc.tile_pool(name="pw", bufs=2, space="PSUM"))

        # --- resident constants (gpsimd DMA queue: keep sync queue for x) ---
        wqk_sb = consts.tile([P, CCH, 2 * C], f32r)
        nc.gpsimd.dma_start(wqk_sb[:], wqk_d[:])
        wvt_sb = consts.tile([P, NPAIR, C], f32r)
        nc.gpsimd.dma_start(wvt_sb[:], wvt_d[:])
        wp_sb = consts.tile([P, NPAIR, C], f32r)
        nc.gpsimd.dma_start(wp_sb[:], wp_d[:])
        ident = consts.tile([P, P], f32)
        make_identity(nc, ident[:])
        zero128 = consts.tile([P, P], f32)
        nc.vector.memset(zero128[:], 0.0)
        # ident_off: rows 64+j have 1 at col j (for building diag(s_k))
        ioff = consts.tile([P, D], f32)
        nc.gpsimd.memset(ioff[:], 0.0)
        nc.gpsimd.affine_select(
            out=ioff[:], in_=ioff[:], compare_op=mybir.AluOpType.not_equal,
            fill=1.0, base=-D, pattern=[[-1, D]], channel_multiplier=1,
        )
        # es_sb[p, h] = exp(scale_h) on the q half (p < 64), 1.0 on the k half
        es_sb = consts.tile([P, HEADS], f32)
        nc.gpsimd.memset(es_sb[D:P, :], 1.0)
        for h in range(HEADS):
            nc.gpsimd.memset(es_sb[0:D, h:h + 1], es[h])
        if add_bv:
            bv_sb = consts.tile([P, NPAIR], f32)
            nc.gpsimd.dma_start(out=bv_sb[:], in_=bv_d[:])
        if add_bp:
            bp_sb = consts.tile([P, CCH], f32)
            nc.gpsimd.dma_start(out=bp_sb[:], in_=bp_d[:])

        for b in range(nb):
            x_r = x_d[b].rearrange("(nt p) c -> p nt c", p=P)
            xt_r = xt_d[b].rearrange("(co p) n -> p co n", p=P)
            yt_r = yt_d[b].rearrange("(ft p) n -> p ft n", p=P)

            # ---- phase 1: G = x^T x  (accumulate over token tiles) ----
            gps = [pgram.tile([P, C], f32, tag="g", name=f"g{b}_{co}")
                   for co in range(CCH)]
            for dd in range(ndma):
                x_t = x_pool.tile([P, 4, C], f32r, tag="x")
                nc.sync.dma_start(out=x_t[:], in_=x_r[:, dd * 4:(dd + 1) * 4, :])
                for tt in range(4):
                    t = dd * 4 + tt
                    for co in range(CCH):
                        nc.tensor.matmul(
                            gps[co][:],
                            x_t[:, tt, co * P:(co + 1) * P],
                            x_t[:, tt, :],
                            start=(t == 0), stop=(t == nt - 1),
                        )
            g_sb = g_pool.tile([P, CCH, C], f32r, tag="g")
            for co in range(CCH):
                nc.vector.tensor_copy(out=g_sb[:, co, :], in_=gps[co][:])

            # ---- phase 2: T = G @ W_qk ----
            t_sb = t_pool.tile([P, CCH, 2 * C], f32r, tag="t")
            for fc in range(2):
                for j in range(CCH):
                    pt = pmm.tile([P, C], f32, tag="pm")
                    for i in range(CCH):
                        nc.tensor.matmul(
                            pt[:],
                            g_sb[:, i, j * P:(j + 1) * P],
                            wqk_sb[:, i, fc * C:(fc + 1) * C],
                            start=(i == 0), stop=(i == CCH - 1),
                        )
                    nc.vector.tensor_copy(
                        out=t_sb[:, j, fc * C:(fc + 1) * C], in_=pt[:])

            # ---- phase 3a: A_h = Gram of [q_h|k_h] for all heads (PE) ----
            a_sb = a_pool.tile([P, HEADS, P], f32, tag="A")
            for h in range(HEADS):
                pa = pmm.tile([P, P], f32, tag="pm")
                for i in range(CCH):
                    nc.tensor.matmul(
                        pa[:],
                        wqk_sb[:, i, h * P:(h + 1) * P],
                        t_sb[:, i, h * P:(h + 1) * P],
                        start=(i == 0), stop=(i == CCH - 1),
                    )
                if add_acorr:
                    ac = ac_pool.tile([P, P], f32, tag="ac")
                    nc.sync.dma_start(out=ac[:], in_=acorr_d[b, h])
                    nc.vector.tensor_add(out=a_sb[:, h, :], in0=pa[:], in1=ac[:])
                else:
                    nc.vector.tensor_copy(out=a_sb[:, h, :], in_=pa[:])

            # ---- phase 3b: l2 norms for all heads (batched DVE/ACT) ----
            s_all = sm_pool.tile([P, HEADS], f32, tag="s")
            for h in range(HEADS):
                dtmp = sm_pool.tile([P, P], f32, tag="dtmp")
                nc.vector.tensor_mul(dtmp[:], a_sb[:, h, :], ident[:])
                nc.vector.reduce_sum(out=s_all[:, h:h + 1], in_=dtmp[:], axis=X)
            nc.vector.tensor_scalar_max(out=s_all[:], in0=s_all[:], scalar1=EPS)
            srt = sm_pool.tile([P, HEADS], f32, tag="srt")
            nc.scalar.activation(out=srt[:], in_=s_all[:], func=AF.Sqrt)
            rs = sm_pool.tile([P, HEADS], f32, tag="rs")
            nc.vector.reciprocal(out=rs[:], in_=srt[:])
            # fold exp(scale_h) into the q-side reciprocal norms
            nc.vector.tensor_mul(rs[:], rs[:], es_sb[:])
            # diag(s_k) at partition base 64, per head
            dsk = sm_pool.tile([P, HEADS, D], f32, tag="dsk")
            for h in range(HEADS):
                nc.vector.tensor_scalar_mul(
                    out=dsk[D:P, h, :], in0=ioff[D:P, :],
                    scalar1=rs[D:P, h:h + 1])

            # ---- phase 3c: attn_pre[dd, e] = (q^T k)[dd, e]*s_k[e] (PE) ----
            pa2 = pmm.tile([D, HEADS * D], f32, tag="pm")
            for h in range(HEADS):
                nc.tensor.matmul(
                    pa2[:, h * D:(h + 1) * D],
                    a_sb[D:P, h, 0:D],
                    dsk[D:P, h, :],
                    start=True, stop=True,
                )

            # ---- phase 3d: softmax (batched; single Exp table load) ----
            asb = sm_pool.tile([D, HEADS, D], f32, tag="asb")
            for h in range(HEADS):
                nc.vector.tensor_scalar_mul(
                    out=asb[:, h, :], in0=pa2[:, h * D:(h + 1) * D],
                    scalar1=rs[0:D, h:h + 1])
            nm = sm_pool.tile([D, HEADS], f32, tag="nm")
            for h in range(HEADS):
                nc.vector.tensor_reduce(
                    out=nm[:, h:h + 1], in_=asb[:, h, :],
                    op=mybir.AluOpType.max, axis=X, negate=True)
            ex = sm_pool.tile([D, HEADS, D], f32, tag="ex")
            zsum = sm_pool.tile([D, HEADS], f32, tag="zsum")
            for h in range(HEADS):
                nc.scalar.activation(
                    out=ex[:, h, :], in_=asb[:, h, :], func=AF.Exp,
                    bias=nm[:, h:h + 1], scale=1.0,
                    accum_out=zsum[:, h:h + 1])
            rinv = sm_pool.tile([D, HEADS], f32, tag="rinv")
            nc.vector.reciprocal(out=rinv[:], in_=zsum[:])

            # ---- phase 3e: U_pair = attn_pair^T @ W_p_pair ----
            u_sbs = []
            for g in range(NPAIR):
                tin = sm_pool.tile([P, P], f32r, tag="tin")
                nc.vector.tensor_copy(out=tin[:], in_=zero128[:])
                for hh in range(2):
                    h = 2 * g + hh
                    nc.vector.tensor_scalar_mul(
                        out=tin[hh * D:(hh + 1) * D, hh * D:(hh + 1) * D],
                        in0=ex[:, h, :], scalar1=rinv[0:D, h:h + 1])
                pu = pmm.tile([P, C], f32, tag="pm")
                nc.tensor.matmul(
                    pu[:], tin[:], wp_sb[:, g, :], start=True, stop=True)
                u_sb = u_pool.tile([P, C], f32r, tag="u", name=f"u{b}_{g}")
                nc.vector.tensor_copy(out=u_sb[:], in_=pu[:])
                u_sbs.append(u_sb)

            # ---- phase 4: W_eff = sum_g W_v_pair^T-stack @ U_pair ----
            weff_sb = w_pool.tile([P, CCH, C], bf16, tag="weff")
            for j in range(CCH):
                pwj = pw.tile([P, C], f32, tag="pw")
                for g in range(NPAIR):
                    nc.tensor.matmul(
                        pwj[:],
                        wvt_sb[:, g, j * P:(j + 1) * P],
                        u_sbs[g][:],
                        start=(g == 0), stop=(g == NPAIR - 1),
                    )
                nc.vector.tensor_copy(out=weff_sb[:, j, :], in_=pwj[:])

            # bias row r[f] = sum_e bv[e] U[e, f] (+ proj_b): gated
            if add_bv or add_bp:
                rfull = sm_pool.tile([P, CCH], f32, tag="rf")
                if add_bv:
                    pr = pmm.tile([P, CCH], f32, tag="pm")
                    for jf in range(CCH):
                        for g in range(NPAIR):
                            nc.tensor.matmul(
                                pr[:, jf:jf + 1],
                                u_sbs[g][:, jf * P:(jf + 1) * P],
                                bv_sb[:, g:g + 1],
                                start=(g == 0), stop=(g == NPAIR - 1),
                            )
                    if add_bp:
                        nc.vector.tensor_add(
                            out=rfull[:], in0=pr[:], in1=bp_sb[:])
                    else:
                        nc.vector.tensor_copy(out=rfull[:], in_=pr[:])
                else:
                    nc.vector.tensor_copy(out=rfull[:], in_=bp_sb[:])

            # ---- phase 5: y^T = W_eff^T @ x^T ----
            for ch in range(nch):
                xT_t = xt_pool.tile([P, CCH, 512], bf16, tag="xT")
                nc.sync.dma_start(
                    out=xT_t[:], in_=xt_r[:, :, ch * 512:(ch + 1) * 512])
                y_sb = y_pool.tile([P, CCH, 512], f32, tag="y")
                for ft in range(CCH):
                    py = pmm.tile([P, 512], f32, tag="pm")
                    for i in range(CCH):
                        nc.tensor.matmul(
                            py[:],
                            weff_sb[:, i, ft * P:(ft + 1) * P],
                            xT_t[:, i, :],
                            start=(i == 0), stop=(i == CCH - 1),
                        )
                    if add_bv or add_bp:
                        nc.vector.tensor_scalar(
                            out=y_sb[:, ft, :], in0=py[:],
                            scalar1=rfull[:, ft:ft + 1], scalar2=None,
                            op0=mybir.AluOpType.add)
                    else:
                        nc.vector.tensor_copy(out=y_sb[:, ft, :], in_=py[:])
                nc.sync.dma_start(
                    out=yt_r[:, :, ch * 512:(ch + 1) * 512], in_=y_sb[:])

    nc.compile()
    return nc


def _get_nc(nb, n, es, add_acorr, add_bv, add_bp):
    key = (nb, n, es, add_acorr, add_bv, add_bp)
    if key not in _CACHE:
        _CACHE[key] = _build(nb, n, es, add_acorr, add_bv, add_bp)
    return _CACHE[key]


def prep_inputs(x, qkv_w, q_bias, v_bias, scale, proj_w, proj_b,
                n_cores=N_CORES):
    """Host-side shard + layout prep. Returns (in_maps, es, gates, meta)."""
    import ml_dtypes

    B, H, W, Cc = x.shape
    assert Cc == C
    n = H * W
    nb = B // n_cores

    xf = np.asarray(x, np.float32).reshape(B, n, C)
    # channel-major bf16 copy for the y = x @ W_eff pass
    xt = np.ascontiguousarray(xf.transpose(0, 2, 1)).astype(ml_dtypes.bfloat16)

    w3 = np.asarray(qkv_w, np.float32).reshape(C, HEADS, 3, D)
    wqk = np.ascontiguousarray(w3[:, :, 0:2, :].reshape(C, 2 * C))
    wqk_r = np.ascontiguousarray(
        wqk.reshape(CCH, P, 2 * C).transpose(1, 0, 2))
    wv = w3[:, :, 2, :].reshape(C, C)
    wvt_r = np.ascontiguousarray(
        np.ascontiguousarray(wv.T).reshape(NPAIR, P, C).transpose(1, 0, 2))
    wp_r = np.ascontiguousarray(
        np.asarray(proj_w, np.float32).reshape(NPAIR, P, C).transpose(1, 0, 2))

    # biases exactly as the reference applies them: concat([q_bias, 0, v_bias])
    # indexed by the raw qkv feature id
    bias_full = np.concatenate(
        [q_bias, np.zeros_like(q_bias), v_bias]).astype(np.float32)
    b3 = bias_full.reshape(HEADS, 3, D)
    bqk = b3[:, 0:2, :].reshape(HEADS, P)      # per-head [q|k] bias
    bv = b3[:, 2, :].reshape(C)                # v bias, (h, d) indexed
    bp = np.asarray(proj_b, np.float32).reshape(C)

    add_acorr = bool(np.any(bqk))
    add_bv = bool(np.any(bv))
    add_bp = bool(np.any(bp))
    es = tuple(float(v) for v in
               np.exp(np.asarray(scale, np.float32)).reshape(HEADS))

    acorr = None
    if add_acorr:
        # A_h correction for qkv bias: with z = [q|k] = x W_h + 1 b_h^T,
        # z^T z = W^T G W + b m^T + m b^T + N b b^T,  m = (1^T x) W_h
        sx = xf.sum(axis=1)                    # [B, C]
        wqk_h = wqk.reshape(C, HEADS, P)       # [c, h, f]
        m = np.einsum("bc,chf->bhf", sx, wqk_h)  # [B, HEADS, 128]
        acorr = (m[:, :, None, :] * bqk[None, :, :, None]
                 + m[:, :, :, None] * bqk[None, :, None, :]
                 + float(n) * bqk[None, :, :, None] * bqk[None, :, None, :]
                 ).astype(np.float32)          # [B, HEADS, 128, 128]
    bv_t = np.ascontiguousarray(
        bv.reshape(NPAIR, P).T).astype(np.float32) if add_bv else None
    bp_t = np.ascontiguousarray(
        bp.reshape(CCH, P).T).astype(np.float32) if add_bp else None

    in_maps = []
    for core in range(n_cores):
        sl = slice(core * nb, (core + 1) * nb)
        m_ = {
            "x": np.ascontiguousarray(xf[sl]),
            "xt": np.ascontiguousarray(xt[sl]),
            "wqk": wqk_r, "wvt": wvt_r, "wp": wp_r,
        }
        if add_acorr:
            m_["acorr"] = np.ascontiguousarray(acorr[sl])
        if add_bv:
            m_["bv"] = bv_t
        if add_bp:
            m_["bp"] = bp_t
        in_maps.append(m_)
    return in_maps, es, (add_acorr, add_bv, add_bp), (B, H, W, nb, n)


def kernel(x, qkv_w, q_bias, v_bias, scale, proj_w, proj_b):
    from concourse.bass_utils import run_bass_kernel_spmd

    in_maps, es, gates, (B, H, W, nb, n) = prep_inputs(
        x, qkv_w, q_bias, v_bias, scale, proj_w, proj_b)
    nc = _get_nc(nb, n, es, *gates)
    res = run_bass_kernel_spmd(
        nc, in_maps, core_ids=list(range(N_CORES)),
        trace=bool(int(os.environ.get("KERNEL_TRACE", "0"))),
    )
    yt = np.concatenate([r["yt"] for r in res.results], axis=0)  # [B, C, n]
    out = np.ascontiguousarray(yt.transpose(0, 2, 1)).reshape(B, H, W, C)
    kernel.last_results = res
    return out.astype(np.float32, copy=False)


# revision 12
# speedup vs baseline: 2.1807x; 1.1389x over previous
"""ChannelAttention Trainium2 kernel (self-contained).

Problem: B=16, H=W=64 (N=4096 tokens), C=512, heads=8, d=64, fp32.
  qkv = x @ qkv_w (+bias);  q,k l2-normalized over tokens;
  attn = softmax((q*exp(scale))^T k);  out = attn @ v^T;  y = out @ proj_w + b.

Sharding: pure data-parallel, 2 batches per core on 8 cores. No collectives.

Algorithm (per batch) — restructured to halve the matmul FLOPs vs the
direct formulation:
  1. G = x^T x                      [C, C]    (contract over N tokens)
  2. T = G @ W_qk                   [C, 2C]   (W_qk = per-head [q|k] columns)
  3. A_h = W_qk_h^T T_h             [128,128] per head = Gram of [q_h|k_h]
     -> diag gives the l2 norms, off-diag block gives q^T k.
  4. W_eff = sum_h W_v_h attn_h^T W_p_h   [C, C]  (head-pair-stacked matmuls)
  5. y^T = W_eff^T x^T              [C, N]   (host transposes back)
Biases (zero in this problem) are handled via gated correction terms.

The two batches are software-pipelined in emission order (the PE executes
in program order): batch b's DVE/ACT softmax section is covered by batch
b+1's G matmuls, and the tiny mid-softmax PE ops (pa2) are spliced into
the middle of the other batch's long matmul phases.

Layouts: x ships twice — token-major bf16 (for G) and channel-major bf16
(for step 5). y returns transposed bf16; host upcasts + transposes back.
"""

import os
import numpy as np

P = 128
C = 512
CCH = C // P            # 4 channel tiles
HEADS = 8
NPAIR = HEADS // 2      # 4 head pairs
D = 64
EPS = 1.55e-5
N_CORES = 8

_CACHE = {}


def _build(nb, n, es, add_acorr, add_bv, add_bp):
    """Build + compile the per-core Bass kernel.

    nb: batches per core; n: tokens per batch; es: tuple of 8 python floats
    (exp(scale), baked); add_*: whether bias corrections are emitted.
    """
    from contextlib import ExitStack
    import concourse.bass as bass  # noqa: F401  (registers engine classes)
    from concourse import bacc
    import concourse.mybir as mybir
    import concourse.tile as tile
    from concourse.masks import make_identity

    f32 = mybir.dt.float32
    f32r = mybir.dt.float32r
    bf16 = mybir.dt.bfloat16
    X = mybir.AxisListType.X
    AF = mybir.ActivationFunctionType

    nt = n // P             # token tiles per batch (32)
    ndma = nt // 4          # x DMAs per batch (4 token tiles each)
    nch = n // 512          # 512-token chunks per batch (8)

    nc = bacc.Bacc("TRN2", target_bir_lowering=False)

    x_d = nc.dram_tensor("x", [nb, n, C], bf16, kind="ExternalInput")
    xt_d = nc.dram_tensor("xt", [nb, C, n], bf16, kind="ExternalInput")
    wqk_d = nc.dram_tensor("wqk", [P, CCH, 2 * C], f32r, kind="ExternalInput")
    wvt_d = nc.dram_tensor("wvt", [P, NPAIR, C], f32r, kind="ExternalInput")
    wp_d = nc.dram_tensor("wp", [P, NPAIR, C], f32r, kind="ExternalInput")
    yt_d = nc.dram_tensor("yt", [nb, C, n], bf16, kind="ExternalOutput")
    if add_acorr:
        acorr_d = nc.dram_tensor(
            "acorr", [nb, HEADS, P, P], f32, kind="ExternalInput")
    if add_bv:
        bv_d = nc.dram_tensor("bv", [P, NPAIR], f32, kind="ExternalInput")
    if add_bp:
        bp_d = nc.dram_tensor("bp", [P, CCH], f32, kind="ExternalInput")

    with tile.TileContext(nc) as tc, ExitStack() as ctx:
        consts = ctx.enter_context(tc.tile_pool(name="consts", bufs=1))
        x_pool = ctx.enter_context(tc.tile_pool(name="xp", bufs=3))
        xt_pool = ctx.enter_context(tc.tile_pool(name="xtp", bufs=3))
        g_pool = ctx.enter_context(tc.tile_pool(name="gp", bufs=2))
        t_pool = ctx.enter_context(tc.tile_pool(name="tp", bufs=2))
        u_pool = ctx.enter_context(tc.tile_pool(name="up", bufs=NPAIR + 1))
        w_pool = ctx.enter_context(tc.tile_pool(name="wp", bufs=2))
        y_pool = ctx.enter_context(tc.tile_pool(name="yp", bufs=3))
        sm_pool = ctx.enter_context(tc.tile_pool(name="smp", bufs=2))
        a_pool = ctx.enter_context(tc.tile_pool(name="ap", bufs=2))
        ac_pool = ctx.enter_context(tc.tile_pool(name="acp", bufs=2))
        pgram = ctx.enter_context(tc.tile_pool(name="pgram", bufs=4, space="PSUM"))
        pmm = ctx.enter_context(tc.tile_pool(name="pmm", bufs=2, space="PSUM"))
        pw = ctx.enter_context(tc.tile_pool(name="pw", bufs=2, space="PSUM"))

        # --- resident constants (gpsimd DMA queue: keep sync queue for x) ---
        wqk_sb = consts.tile([P, CCH, 2 * C], f32r)
        nc.gpsimd.dma_start(wqk_sb[:], wqk_d[:])
        wvt_sb = consts.tile([P, NPAIR, C], f32r)
        nc.gpsimd.dma_start(wvt_sb[:], wvt_d[:])
        wp_sb = consts.tile([P, NPAIR, C], f32r)
        nc.gpsimd.dma_start(wp_sb[:], wp_d[:])
        ident = consts.tile([P, P], f32)
        make_identity(nc, ident[:])
        zero128 = consts.tile([P, P], f32)
        nc.vector.memset(zero128[:], 0.0)
        # ident_off: rows 64+j have 1 at col j (for building diag(s_k))
        ioff = consts.tile([P, D], f32)
        nc.gpsimd.memset(ioff[:], 0.0)
        nc.gpsimd.affine_select(
            out=ioff[:], in_=ioff[:], compare_op=mybir.AluOpType.not_equal,
            fill=1.0, base=-D, pattern=[[-1, D]], channel_multiplier=1,
        )
        # es_sb[p, h] = exp(scale_h) on the q half (p < 64), 1.0 on the k half
        es_sb = consts.tile([P, HEADS], f32)
        nc.gpsimd.memset(es_sb[D:P, :], 1.0)
        for h in range(HEADS):
            nc.gpsimd.memset(es_sb[0:D, h:h + 1], es[h])
        if add_bv:
            bv_sb = consts.tile([P, NPAIR], f32)
            nc.gpsimd.dma_start(out=bv_sb[:], in_=bv_d[:])
        if add_bp:
            bp_sb = consts.tile([P, CCH], f32)
            nc.gpsimd.dma_start(out=bp_sb[:], in_=bp_d[:])

        # per-batch state carried between pipeline stages
        st = [dict() for _ in range(nb)]

        def em_G(b, mid_emit=None):
            """G = x^T x; mid_emit() spliced after the first 2 DMA groups."""
            x_r = x_d[b].rearrange("(nt p) c -> p nt c", p=P)
            gps = [pgram.tile([P, C], f32, tag="g", name=f"g{b}_{co}")
                   for co in range(CCH)]
            for dd in range(ndma):
                x_t = x_pool.tile([P, 4, C], bf16, tag="x")
                nc.sync.dma_start(
                    out=x_t[:], in_=x_r[:, dd * 4:(dd + 1) * 4, :])
                for tt in range(4):
                    t = dd * 4 + tt
                    for co in range(CCH):
                        nc.tensor.matmul(
                            gps[co][:],
                            x_t[:, tt, co * P:(co + 1) * P],
                            x_t[:, tt, :],
                            start=(t == 0), stop=(t == nt - 1),
                        )
                if dd == 1 and mid_emit is not None:
                    mid_emit()
            g_sb = g_pool.tile([P, CCH, C], f32r, tag="g")
            for co in range(CCH):
                nc.vector.tensor_copy(out=g_sb[:, co, :], in_=gps[co][:])
            st[b]["g_sb"] = g_sb

        def em_T(b):
            g_sb = st[b]["g_sb"]
            t_sb = t_pool.tile([P, CCH, 2 * C], f32r, tag="t")
            for fc in range(2):
                for j in range(CCH):
                    pt = pmm.tile([P, C], f32, tag="pm")
                    for i in range(CCH):
                        nc.tensor.matmul(
                            pt[:],
                            g_sb[:, i, j * P:(j + 1) * P],
                            wqk_sb[:, i, fc * C:(fc + 1) * C],
                            start=(i == 0), stop=(i == CCH - 1),
                        )
                    nc.vector.tensor_copy(
                        out=t_sb[:, j, fc * C:(fc + 1) * C], in_=pt[:])
            st[b]["t_sb"] = t_sb

        def em_A(b):
            t_sb = st[b]["t_sb"]
            a_sb = a_pool.tile([P, HEADS, P], f32, tag="A")
            for h in range(HEADS):
                pa = pmm.tile([P, P], f32, tag="pm")
                for i in range(CCH):
                    nc.tensor.matmul(
                        pa[:],
                        wqk_sb[:, i, h * P:(h + 1) * P],
                        t_sb[:, i, h * P:(h + 1) * P],
                        start=(i == 0), stop=(i == CCH - 1),
                    )
                if add_acorr:
                    ac = ac_pool.tile([P, P], f32, tag="ac")
                    nc.sync.dma_start(out=ac[:], in_=acorr_d[b, h])
                    nc.vector.tensor_add(
                        out=a_sb[:, h, :], in0=pa[:], in1=ac[:])
                else:
                    nc.vector.tensor_copy(out=a_sb[:, h, :], in_=pa[:])
            st[b]["a_sb"] = a_sb

        def em_norms(b):
            """l2 norms for all heads + diag(s_k) prep (DVE/ACT, batched)."""
            a_sb = st[b]["a_sb"]
            s_all = sm_pool.tile([P, HEADS], f32, tag="s")
            for h in range(HEADS):
                dtmp = sm_pool.tile([P, P], f32, tag="dtmp")
                nc.vector.tensor_mul(dtmp[:], a_sb[:, h, :], ident[:])
                nc.vector.reduce_sum(
                    out=s_all[:, h:h + 1], in_=dtmp[:], axis=X)
            nc.vector.tensor_scalar_max(
                out=s_all[:], in0=s_all[:], scalar1=EPS)
            srt = sm_pool.tile([P, HEADS], f32, tag="srt")
            nc.scalar.activation(out=srt[:], in_=s_all[:], func=AF.Sqrt)
            rs = sm_pool.tile([P, HEADS], f32, tag="rs")
            nc.vector.reciprocal(out=rs[:], in_=srt[:])
            # fold exp(scale_h) into the q-side reciprocal norms
            nc.vector.tensor_mul(rs[:], rs[:], es_sb[:])
            dsk = sm_pool.tile([P, HEADS, D], f32, tag="dsk")
            for h in range(HEADS):
                nc.vector.tensor_scalar_mul(
                    out=dsk[D:P, h, :], in0=ioff[D:P, :],
                    scalar1=rs[D:P, h:h + 1])
            st[b]["rs"] = rs
            st[b]["dsk"] = dsk

        def em_pa2(b):
            """attn_pre[dd, e] = (q^T k)[dd, e]*s_k[e]  (tiny PE matmuls)."""
            a_sb, dsk = st[b]["a_sb"], st[b]["dsk"]
            pa2 = pw.tile([D, HEADS * D], f32, tag="pw", name=f"pa2_{b}")
            for h in range(HEADS):
                nc.tensor.matmul(
                    pa2[:, h * D:(h + 1) * D],
                    a_sb[D:P, h, 0:D],
                    dsk[D:P, h, :],
                    start=True, stop=True,
                )
            st[b]["pa2"] = pa2

        def em_soft(b):
            """softmax over e for all heads (DVE/ACT, batched)."""
            rs, pa2 = st[b]["rs"], st[b]["pa2"]
            asb = sm_pool.tile([D, HEADS, D], f32, tag="asb")
            for h in range(HEADS):
                nc.vector.tensor_scalar_mul(
                    out=asb[:, h, :], in0=pa2[:, h * D:(h + 1) * D],
                    scalar1=rs[0:D, h:h + 1])
            nm = sm_pool.tile([D, HEADS], f32, tag="nm")
            for h in range(HEADS):
                nc.vector.tensor_reduce(
                    out=nm[:, h:h + 1], in_=asb[:, h, :],
                    op=mybir.AluOpType.max, axis=X, negate=True)
            ex = sm_pool.tile([D, HEADS, D], f32, tag="ex")
            zsum = sm_pool.tile([D, HEADS], f32, tag="zsum")
            for h in range(HEADS):
                nc.scalar.activation(
                    out=ex[:, h, :], in_=asb[:, h, :], func=AF.Exp,
                    bias=nm[:, h:h + 1], scale=1.0,
                    accum_out=zsum[:, h:h + 1])
            rinv = sm_pool.tile([D, HEADS], f32, tag="rinv")
            nc.vector.reciprocal(out=rinv[:], in_=zsum[:])
            st[b]["ex"] = ex
            st[b]["rinv"] = rinv

        def em_U(b):
            """U_pair = attn_pair^T @ W_p_pair for all pairs."""
            ex, rinv = st[b]["ex"], st[b]["rinv"]
            u_sbs = []
            for g in range(NPAIR):
                tin = sm_pool.tile([P, P], f32r, tag="tin")
                nc.vector.tensor_copy(out=tin[:], in_=zero128[:])
                for hh in range(2):
                    h = 2 * g + hh
                    nc.vector.tensor_scalar_mul(
                        out=tin[hh * D:(hh + 1) * D, hh * D:(hh + 1) * D],
                        in0=ex[:, h, :], scalar1=rinv[0:D, h:h + 1])
                pu = pmm.tile([P, C], f32, tag="pm")
                nc.tensor.matmul(
                    pu[:], tin[:], wp_sb[:, g, :], start=True, stop=True)
                u_sb = u_pool.tile([P, C], f32r, tag="u", name=f"u{b}_{g}")
                nc.vector.tensor_copy(out=u_sb[:], in_=pu[:])
                u_sbs.append(u_sb)
            st[b]["u_sbs"] = u_sbs

        def em_Weff(b):
            u_sbs = st[b]["u_sbs"]
            weff_sb = w_pool.tile([P, CCH, C], bf16, tag="weff")
            for j in range(CCH):
                pwj = pw.tile([P, C], f32, tag="pw")
                for g in range(NPAIR):
                    nc.tensor.matmul(
                        pwj[:],
                        wvt_sb[:, g, j * P:(j + 1) * P],
                        u_sbs[g][:],
                        start=(g == 0), stop=(g == NPAIR - 1),
                    )
                nc.vector.tensor_copy(out=weff_sb[:, j, :], in_=pwj[:])
            st[b]["weff_sb"] = weff_sb
            # bias row r[f] = sum_e bv[e] U[e, f] (+ proj_b): gated
            if add_bv or add_bp:
                rfull = sm_pool.tile([P, CCH], f32, tag="rf")
                if add_bv:
                    pr = pmm.tile([P, CCH], f32, tag="pm")
                    for jf in range(CCH):
                        for g in range(NPAIR):
                            nc.tensor.matmul(
                                pr[:, jf:jf + 1],
                                u_sbs[g][:, jf * P:(jf + 1) * P],
                                bv_sb[:, g:g + 1],
                                start=(g == 0), stop=(g == NPAIR - 1),
                            )
                    if add_bp:
                        nc.vector.tensor_add(
                            out=rfull[:], in0=pr[:], in1=bp_sb[:])
                    else:
                        nc.vector.tensor_copy(out=rfull[:], in_=pr[:])
                else:
                    nc.vector.tensor_copy(out=rfull[:], in_=bp_sb[:])
                st[b]["rfull"] = rfull

        def em_apply(b, mid_emit=None):
            """y^T = W_eff^T @ x^T; mid_emit() spliced after 2 chunks."""
            weff_sb = st[b]["weff_sb"]
            rfull = st[b].get("rfull")
            xt_r = xt_d[b].rearrange("(co p) n -> p co n", p=P)
            yt_r = yt_d[b].rearrange("(ft p) n -> p ft n", p=P)
            for ch in range(nch):
                xT_t = xt_pool.tile([P, CCH, 512], bf16, tag="xT")
                nc.sync.dma_start(
                    out=xT_t[:], in_=xt_r[:, :, ch * 512:(ch + 1) * 512])
                y_sb = y_pool.tile([P, CCH, 512], bf16, tag="y")
                for ft in range(CCH):
                    py = pmm.tile([P, 512], f32, tag="pm")
                    for i in range(CCH):
                        nc.tensor.matmul(
                            py[:],
                            weff_sb[:, i, ft * P:(ft + 1) * P],
                            xT_t[:, i, :],
                            start=(i == 0), stop=(i == CCH - 1),
                        )
                    if rfull is not None:
                        nc.vector.tensor_scalar(
                            out=y_sb[:, ft, :], in0=py[:],
                            scalar1=rfull[:, ft:ft + 1], scalar2=None,
                            op0=mybir.AluOpType.add)
                    else:
                        nc.vector.tensor_copy(out=y_sb[:, ft, :], in_=py[:])
                nc.sync.dma_start(
                    out=yt_r[:, :, ch * 512:(ch + 1) * 512], in_=y_sb[:])
                if ch == 1 and mid_emit is not None:
                    mid_emit()

        # ---- software-pipelined emission over batches ----
        for b in range(nb):
            em_G(b, mid_emit=(lambda bb=b - 1: em_pa2(bb)) if b > 0 else None)
            if b > 0:
                em_soft(b - 1)
                em_U(b - 1)
                em_Weff(b - 1)
            em_T(b)
            em_A(b)
            em_norms(b)
            if b > 0:
                em_apply(b - 1, mid_emit=(lambda bb=b: em_pa2(bb)))
        last = nb - 1
        if nb == 1:
            em_pa2(last)
        em_soft(last)
        em_U(last)
        em_Weff(last)
        em_apply(last)

    nc.compile()
    return nc


def _get_nc(nb, n, es, add_acorr, add_bv, add_bp):
    key = (nb, n, es, add_acorr, add_bv, add_bp)
    if key not in _CACHE:
        _CACHE[key] = _build(nb, n, es, add_acorr, add_bv, add_bp)
    return _CACHE[key]


def prep_inputs(x, qkv_w, q_bias, v_bias, scale, proj_w, proj_b,
                n_cores=N_CORES):
    """Host-side shard + layout prep. Returns (in_maps, es, gates, meta)."""
    import ml_dtypes

    B, H, W, Cc = x.shape
    assert Cc == C
    n = H * W
    nb = B // n_cores

    xf = np.asarray(x, np.float32).reshape(B, n, C)
    xbf = xf.astype(ml_dtypes.bfloat16)
    # channel-major bf16 copy for the y = x @ W_eff pass
    xt = np.ascontiguousarray(xbf.transpose(0, 2, 1))

    w3 = np.asarray(qkv_w, np.float32).reshape(C, HEADS, 3, D)
    wqk = np.ascontiguousarray(w3[:, :, 0:2, :].reshape(C, 2 * C))
    wqk_r = np.ascontiguousarray(
        wqk.reshape(CCH, P, 2 * C).transpose(1, 0, 2))
    wv = w3[:, :, 2, :].reshape(C, C)
    wvt_r = np.ascontiguousarray(
        np.ascontiguousarray(wv.T).reshape(NPAIR, P, C).transpose(1, 0, 2))
    wp_r = np.ascontiguousarray(
        np.asarray(proj_w, np.float32).reshape(NPAIR, P, C).transpose(1, 0, 2))

    # biases exactly as the reference applies them: concat([q_bias, 0, v_bias])
    # indexed by the raw qkv feature id
    bias_full = np.concatenate(
        [q_bias, np.zeros_like(q_bias), v_bias]).astype(np.float32)
    b3 = bias_full.reshape(HEADS, 3, D)
    bqk = b3[:, 0:2, :].reshape(HEADS, P)      # per-head [q|k] bias
    bv = b3[:, 2, :].reshape(C)                # v bias, (h, d) indexed
    bp = np.asarray(proj_b, np.float32).reshape(C)

    add_acorr = bool(np.any(bqk))
    add_bv = bool(np.any(bv))
    add_bp = bool(np.any(bp))
    es = tuple(float(v) for v in
               np.exp(np.asarray(scale, np.float32)).reshape(HEADS))

    acorr = None
    if add_acorr:
        # A_h correction for qkv bias: with z = [q|k] = x W_h + 1 b_h^T,
        # z^T z = W^T G W + b m^T + m b^T + N b b^T,  m = (1^T x) W_h
        sx = xf.sum(axis=1)                    # [B, C]
        wqk_h = wqk.reshape(C, HEADS, P)       # [c, h, f]
        m = np.einsum("bc,chf->bhf", sx, wqk_h)  # [B, HEADS, 128]
        acorr = (m[:, :, None, :] * bqk[None, :, :, None]
                 + m[:, :, :, None] * bqk[None, :, None, :]
                 + float(n) * bqk[None, :, :, None] * bqk[None, :, None, :]
                 ).astype(np.float32)          # [B, HEADS, 128, 128]
    bv_t = np.ascontiguousarray(
        bv.reshape(NPAIR, P).T).astype(np.float32) if add_bv else None
    bp_t = np.ascontiguousarray(
        bp.reshape(CCH, P).T).astype(np.float32) if add_bp else None

    in_maps = []
    for core in range(n_cores):
        sl = slice(core * nb, (core + 1) * nb)
        m_ = {
            "x": np.ascontiguousarray(xbf[sl]),
            "xt": np.ascontiguousarray(xt[sl]),
            "wqk": wqk_r, "wvt": wvt_r, "wp": wp_r,
        }
        if add_acorr:
            m_["acorr"] = np.ascontiguousarray(acorr[sl])
        if add_bv:
            m_["bv"] = bv_t
        if add_bp:
            m_["bp"] = bp_t
        in_maps.append(m_)
    return in_maps, es, (add_acorr, add_bv, add_bp), (B, H, W, nb, n)


def kernel(x, qkv_w, q_bias, v_bias, scale, proj_w, proj_b):
    from concourse.bass_utils import run_bass_kernel_spmd

    in_maps, es, gates, (B, H, W, nb, n) = prep_inputs(
        x, qkv_w, q_bias, v_bias, scale, proj_w, proj_b)
    nc = _get_nc(nb, n, es, *gates)
    res = run_bass_kernel_spmd(
        nc, in_maps, core_ids=list(range(N_CORES)),
        trace=bool(int(os.environ.get("KERNEL_TRACE", "0"))),
    )
    yt = np.concatenate([r["yt"] for r in res.results], axis=0)  # [B, C, n]
    out = np.ascontiguousarray(
        yt.astype(np.float32).transpose(0, 2, 1)).reshape(B, H, W, C)
    kernel.last_results = res
    return out
